# revision 1
# baseline (speedup 1.0000x reference)
"""Bahdanau attention Trainium2 kernel.

Problem (per full input):
    query [32, 1024], values [32, 2048, 1024], W1 [1024, 1024],
    W2 [1024, 1024], V [1024, 1]
    q_proj = query @ W1                       # [B, U]
    v_proj = values @ W2                      # [B, T, U]
    score  = tanh(q_proj[:, None] + v_proj)   # [B, T, U]
    logits = score @ V                        # [B, T, 1]
    attn   = softmax(logits, axis=1)          # [B, T, 1]
    ctx    = sum_t attn * values              # [B, D]
    returns (ctx, attn)

Sharding: data-parallel over batch, 4 batches per core on 8 cores.

Per-core plan (u-major "transposed" layout so that the q_proj add is a
free per-partition ACT bias, the logits reduction is a PE matmul, and
the softmax runs along the free axis):
    - weights cast to bf16 on load (SWDGE cast DMA)
    - values: fp32 DRAM -> bf16 DRAM (cast DMA), then
        * xbar DMA transpose  -> valT [128(d), 8, 2048(t)]  (rhs of main mm)
        * straight load       -> vnat [128(t), 16, 1024(d)] (context mm)
    - main mm: vprojT[u, t] += W2bf[d, u].T @ valT[d, t] (8 k-chunks)
    - ACT: scoreT = tanh(vprojT + qprojT[u] bias) -> bf16
    - logits mm: lhsT = Vbf [u, 1] -> logits [1, t]
    - softmax on [1, 2048] row (DVE/ACT, exp accum_out gives sum)
    - context mm: lhsT = attnT [t, 1], rhs = vnat -> ctx [1, d]
"""
import numpy as np
from contextlib import ExitStack

import concourse.bass as bass
import concourse.bacc as bacc
import concourse.tile as tile
from concourse import mybir, masks
from concourse import bass_utils

B, T, D, U = 32, 2048, 1024, 1024
NCORES = 8
BSH = B // NCORES          # 4 batches per core
P = 128
DC = D // P                # 8 contraction chunks
UC = U // P                # 8 u tiles
NF = 512                   # matmul free dim (one PSUM bank of fp32)
TC = T // NF               # 4 t chunks
TT = T // P                # 16 t tiles
f32 = mybir.dt.float32
bf16 = mybir.dt.bfloat16
AF = mybir.ActivationFunctionType


def _body(ctx, tc, values, query, W1, W2, V, ctx_out, attn_out):
    nc = tc.nc
    const = ctx.enter_context(tc.tile_pool(name="const", bufs=1))
    dram = ctx.enter_context(tc.tile_pool(name="dram", bufs=2, space="DRAM"))
    vpool = ctx.enter_context(tc.tile_pool(name="vpool", bufs=2))
    npool = ctx.enter_context(tc.tile_pool(name="npool", bufs=1))
    spool = ctx.enter_context(tc.tile_pool(name="spool", bufs=2))
    smax = ctx.enter_context(tc.tile_pool(name="smax", bufs=2))
    mm_ps = ctx.enter_context(tc.tile_pool(name="mm_ps", bufs=3, space="PSUM"))
    lg_ps = ctx.enter_context(tc.tile_pool(name="lg_ps", bufs=2, space="PSUM"))
    cx_ps = ctx.enter_context(tc.tile_pool(name="cx_ps", bufs=1, space="PSUM"))
    tp_ps = ctx.enter_context(tc.tile_pool(name="tp_ps", bufs=1, space="PSUM"))

    # ---- constants -------------------------------------------------------
    ident = const.tile([P, P], bf16)
    masks.make_identity(nc, ident)

    W2bf = const.tile([P, DC, U], bf16)
    nc.gpsimd.dma_start(W2bf, W2.rearrange("(dc p) u -> p dc u", p=P))
    W1bf = const.tile([P, DC, U], bf16)
    nc.gpsimd.dma_start(W1bf, W1.rearrange("(dc p) u -> p dc u", p=P))
    Vbf = const.tile([P, UC, 1], bf16)
    nc.gpsimd.dma_start(Vbf, V.rearrange("(uc p) o -> p uc o", p=P))

    # ---- qprojT[u, b] = (query @ W1).T ----------------------------------
    q_sb = const.tile([BSH, D], f32)
    nc.sync.dma_start(q_sb, query)
    q_bf = const.tile([BSH, D], bf16)
    nc.vector.tensor_copy(q_bf, q_sb)
    qT = const.tile([P, DC, BSH], bf16)
    for dc in range(DC):
        tp = tp_ps.tile([P, BSH], bf16, tag="tp")
        nc.tensor.transpose(tp, q_bf[:, dc * P:(dc + 1) * P], ident[:BSH, :BSH])
        nc.vector.tensor_copy(qT[:, dc, :], tp)
    qpT = const.tile([P, UC, BSH], f32)
    for uc in range(UC):
        qp = tp_ps.tile([P, BSH], f32, tag="tp")
        for dc in range(DC):
            nc.tensor.matmul(qp, W1bf[:, dc, uc * P:(uc + 1) * P], qT[:, dc, :],
                             start=dc == 0, stop=dc == DC - 1)
        nc.vector.tensor_copy(qpT[:, uc, :], qp)

    # ---- per-batch pipeline ---------------------------------------------
    for b in range(BSH):
        valbf = dram.tile([T, D], bf16, tag="valbf")
        nc.gpsimd.dma_start(valbf, values[b])
        valT = vpool.tile([P, DC, T], bf16, tag="valT")
        nc.sync.dma_start_transpose(valT, valbf)
        vnat = npool.tile([P, TT, D], bf16, tag="vnat")
        nc.sync.dma_start(vnat, valbf.rearrange("(tt p) d -> p tt d", p=P))

        logits = smax.tile([1, T], f32, tag="logits")
        for tc4 in range(TC):
            tsl = slice(tc4 * NF, (tc4 + 1) * NF)
            scoreT = spool.tile([P, UC, NF], bf16, tag="score")
            for uc in range(UC):
                ps = mm_ps.tile([P, NF], f32, tag="mm")
                for dc in range(DC):
                    nc.tensor.matmul(ps,
                                     W2bf[:, dc, uc * P:(uc + 1) * P],
                                     valT[:, dc, tsl],
                                     start=dc == 0, stop=dc == DC - 1)
                nc.scalar.activation(scoreT[:, uc, :], ps, AF.Tanh,
                                     bias=qpT[:, uc, b:b + 1])
            lps = lg_ps.tile([1, NF], f32, tag="lg")
            for uc in range(UC):
                nc.tensor.matmul(lps, Vbf[:, uc, :], scoreT[:, uc, :],
                                 start=uc == 0, stop=uc == UC - 1)
            nc.vector.tensor_copy(logits[:, tsl], lps)

        # softmax over the free axis on one partition
        mx = smax.tile([1, 1], f32, tag="mx")
        nc.vector.reduce_max(mx, logits, axis=mybir.AxisListType.X)
        negmx = smax.tile([1, 1], f32, tag="negmx")
        nc.vector.tensor_scalar_mul(negmx, mx, -1.0)
        probs = smax.tile([1, T], f32, tag="probs")
        sumexp = smax.tile([1, 1], f32, tag="sumexp")
        nc.scalar.activation(probs, logits, AF.Exp, bias=negmx,
                             accum_out=sumexp)
        rse = smax.tile([1, 1], f32, tag="rse")
        nc.vector.reciprocal(rse, sumexp)
        nc.vector.tensor_scalar_mul(probs, probs, rse)
        nc.sync.dma_start(attn_out[b], probs)

        # context: ctx[1, d] = sum_t attn[t] * values[t, d]
        attn16 = smax.tile([1, T], bf16, tag="attn16")
        nc.vector.tensor_copy(attn16, probs)
        attnT = smax.tile([P, TT, 1], bf16, tag="attnT")
        for tt in range(TT):
            tp2 = tp_ps.tile([P, 1], bf16, tag="tp")
            nc.tensor.transpose(tp2, attn16[:, tt * P:(tt + 1) * P],
                                ident[:1, :1])
            nc.vector.tensor_copy(attnT[:, tt, :], tp2)
        cxa = cx_ps.tile([1, NF], f32, tag="cx0")
        cxb = cx_ps.tile([1, NF], f32, tag="cx1")
        for tt in range(TT):
            nc.tensor.matmul(cxa, attnT[:, tt, :], vnat[:, tt, 0:NF],
                             start=tt == 0, stop=tt == TT - 1)
            nc.tensor.matmul(cxb, attnT[:, tt, :], vnat[:, tt, NF:D],
                             start=tt == 0, stop=tt == TT - 1)
        ctx_sb = smax.tile([1, D], f32, tag="ctx_sb")
        nc.vector.tensor_copy(ctx_sb[:, 0:NF], cxa)
        nc.vector.tensor_copy(ctx_sb[:, NF:D], cxb)
        nc.sync.dma_start(ctx_out[b], ctx_sb)


def build():
    nc = bacc.Bacc("TRN2", target_bir_lowering=False, debug=False,
                   num_devices=NCORES)
    values = nc.dram_tensor("values", (BSH, T, D), f32,
                            kind="ExternalInput").ap()
    query = nc.dram_tensor("query", (BSH, D), f32, kind="ExternalInput").ap()
    W1 = nc.dram_tensor("W1", (D, U), f32, kind="ExternalInput").ap()
    W2 = nc.dram_tensor("W2", (D, U), f32, kind="ExternalInput").ap()
    V = nc.dram_tensor("V", (U, 1), f32, kind="ExternalInput").ap()
    ctx_out = nc.dram_tensor("ctx", (BSH, D), f32, kind="ExternalOutput").ap()
    attn_out = nc.dram_tensor("attn", (BSH, T), f32,
                              kind="ExternalOutput").ap()
    with tile.TileContext(nc) as tc, ExitStack() as ctx:
        _body(ctx, tc, values, query, W1, W2, V, ctx_out, attn_out)
    nc.compile()
    return nc


_NC = None


def _get_nc():
    global _NC
    if _NC is None:
        _NC = build()
    return _NC


def _run(in_maps, **kwargs):
    nc = _get_nc()
    return bass_utils.run_bass_kernel_spmd(nc, in_maps,
                                           core_ids=list(range(NCORES)),
                                           **kwargs)


def make_in_maps(query, values, W1, W2, V):
    query = np.ascontiguousarray(np.asarray(query, dtype=np.float32))
    values = np.ascontiguousarray(np.asarray(values, dtype=np.float32))
    W1 = np.ascontiguousarray(np.asarray(W1, dtype=np.float32))
    W2 = np.ascontiguousarray(np.asarray(W2, dtype=np.float32))
    V = np.ascontiguousarray(np.asarray(V, dtype=np.float32))
    in_maps = []
    for c in range(NCORES):
        sl = slice(c * BSH, (c + 1) * BSH)
        in_maps.append({
            "values": np.ascontiguousarray(values[sl]),
            "query": np.ascontiguousarray(query[sl]),
            "W1": W1, "W2": W2, "V": V,
        })
    return in_maps


def assemble(results):
    context = np.concatenate([results[c]["ctx"] for c in range(NCORES)], 0)
    attn = np.concatenate([results[c]["attn"] for c in range(NCORES)], 0)
    return context.astype(np.float32), attn[..., None].astype(np.float32)


def kernel(query, values, W1, W2, V):
    in_maps = make_in_maps(query, values, W1, W2, V)
    res = _run(in_maps)
    return assemble(res.results)


if __name__ == "__main__":
    rng = np.random.default_rng(0)
    scale = 1.0 / np.sqrt(D)
    inputs = dict(
        query=rng.standard_normal((B, D), dtype=np.float32),
        values=rng.standard_normal((B, T, D), dtype=np.float32),
        W1=rng.standard_normal((D, U), dtype=np.float32) * scale,
        W2=rng.standard_normal((D, U), dtype=np.float32) * scale,
        V=rng.standard_normal((U, 1), dtype=np.float32) / np.sqrt(U),
    )
    ctx_np, attn_np = kernel(**inputs)
    print(ctx_np.shape, attn_np.shape, ctx_np.dtype, attn_np.dtype)
